# revision 9
# baseline (speedup 1.0000x reference)
"""BERT-embedding kernel for 8 Trainium2 NeuronCores (Bass/Tile).

out[b,s,:] = concat( input[b,s,:] @ W.T + b_vec,  PE[doy[b,s], :] )
with PE the standard sinusoidal table (d_model=256, max_len=366).

Strategy (data-parallel over batch, 8 cores), v3:
  - core c handles batches [c*128, (c+1)*128) = 16384 tokens.
  - HBM-write-bound: 33.5 MB/core of output; the 16 SDMA engines run at
    ~26.6 GB/s each (97% of the 27.2 GB/s AXI-port limit) once chunks
    are >= 8 KB contiguous per partition. Output DRAM layout is
    partition-contiguous ([128, G*512]; host unscrambles) to get there.
  - obs half: TensorE matmul in bf16 (rel err ~3.5e-3 << 2e-2 gate).
    Tiles 0,1 are single matmuls (K=11) so the first groups can be
    1-2 tiles; tiles 2..127 go two-per-matmul via a block-diagonal
    stationary operand (K = 2*11 = 22, N = 512). lhs is split into two
    DRAM tensors loaded at partition offsets 0 and 64 so input DMAs
    spread over both the even and odd SDMA ports.
  - PE half: sin/cos(doy*div[i]) on ACT. Sin spline valid on [-pi,pi]:
    cols i < R are range-reduced with the f32 magic-number rounding
    trick on DVE. cos(y) = sin(pi/2 - |y|), |y| via ACT Abs ordered
    AFTER the group's Sin so the ramp-critical path stays short. The
    Sin ACT table is warmed at body start to overlap the input DMAs.
  - angle inputs (div table + transposed doy) live in one DRAM tensor,
    loaded in two DMAs (first covers tiles 0..7) for an early first
    completion semaphore.
  - groups ramp [1,1,2,4,4,4] then 8 tiles; groups with t0 < 16 stream
    the PE half from the ACT sequencer (scalar.dma_start, separate HWDGE
    ring) as soon as sin/cos finish, obs half follows on SP, and their
    PSUM copies all go to DVE to keep ACT on sin/abs/cos during the
    pipeline-fill phase.
"""
import numpy as np
import ml_dtypes

# ---------------- problem constants (hardcoded per contract) ----------------
B, S, F, D = 1024, 128, 10, 256
MAX_LEN = 366
N_CORES = 8
BPC = B // N_CORES          # batches per core
TOK = BPC * S               # tokens per core = 16384
P = 128                     # tokens per tile (SBUF partitions)
G = TOK // P                # 128 tiles per core
GROUP_PLAN = [1, 1, 2, 4, 4, 4] + [8] * 14
assert sum(GROUP_PLAN) == G
RAMP_TILES = 16             # groups starting below this stream halves + DVE copies
K = F + 1                   # contraction dim incl. bias row
K2 = 2 * K                  # packed two-tile contraction dim
R = 68                      # columns needing range reduction (365*div[68] < pi)
NPAIR = (G - 2) // 2        # 63 pairs over tiles 2..127
NPAIR_A = 32                # pairs in lhsA; rest in lhsB @ partition 64
SINGLE_OFF = NPAIR_A * P    # lhsA col offset of the two single tiles (0,1)
RHSA_OFF = SINGLE_OFF + 2 * P   # lhsA col offset of the rhs block
NB = NPAIR - NPAIR_A        # 31 pairs in lhsB
RHSB_OFF = NB * P           # lhsB col offset of its rhs copy
DD_SPLIT = 128 + 8          # first dd DMA covers div + doy for tiles 0..7

PI = float(np.float32(np.pi))
HALF_PI = float(np.float32(np.pi / 2))
TWO_PI = float(np.float32(2 * np.pi))
INV_2PI = float(np.float32(1.0 / (2 * np.pi)))
MAGIC = 12582912.0          # 1.5 * 2**23: (x+MAGIC)-MAGIC == round-to-nearest(x)

_CACHE = {}


def _build_nc():
    import concourse.bacc as bacc
    import concourse.tile as tile
    import concourse.mybir as mybir

    F32 = mybir.dt.float32
    BF16 = mybir.dt.bfloat16
    AOT = mybir.AluOpType
    ACT = mybir.ActivationFunctionType

    nc = bacc.Bacc("TRN2", target_bir_lowering=False, debug=False,
                   num_devices=N_CORES)
    lhsA_d = nc.dram_tensor("lhsA", [K2, RHSA_OFF + 2 * D], BF16,
                            kind="ExternalInput")
    lhsB_d = nc.dram_tensor("lhsB", [K2, RHSB_OFF + 2 * D], BF16,
                            kind="ExternalInput")
    # dd: cols 0:128 = div row (per-partition copy), 128:128+G = doyT
    dd_d = nc.dram_tensor("dd", [P, 128 + G], F32, kind="ExternalInput")
    out_d = nc.dram_tensor("out", [P, G * 2 * D], F32, kind="ExternalOutput")

    outv = out_d[:].rearrange("p (t c) -> p t c", c=2 * D)

    with tile.TileContext(nc) as tc:
        with (
            tc.tile_pool(name="const", bufs=1) as cpool,
            tc.tile_pool(name="angp", bufs=3) as angp,
            tc.tile_pool(name="outp", bufs=6) as outp,
            tc.tile_pool(name="psum", bufs=6, space="PSUM") as psump,
            tc.tile_pool(name="psum1", bufs=2, space="PSUM") as psump1,
        ):
            dd_sb = cpool.tile([P, 128 + G], F32)
            nc.sync.dma_start(dd_sb[:, 0:DD_SPLIT], dd_d[:, 0:DD_SPLIT])
            halfpi = cpool.tile([P, 1], F32)
            nc.vector.memset(halfpi[:], HALF_PI)
            # warm the Sin ACT table immediately so ACT_TABLE_LOAD overlaps
            # the input DMAs instead of gating the first PE-half group
            warm = cpool.tile([P, 1], F32)
            nc.scalar.activation(warm[:], halfpi[:], ACT.Sin)
            nc.sync.dma_start(dd_sb[:, DD_SPLIT:], dd_d[:, DD_SPLIT:])
            # matmul inputs: even ports (partitions 0..21) and odd ports
            # (partitions 64..85)
            lhsA_sb = cpool.tile([K2, RHSA_OFF + 2 * D], BF16)
            nc.sync.dma_start(lhsA_sb[:], lhsA_d[:])
            lhsB_sb = cpool.tile([64 + K2, RHSB_OFF + 2 * D], BF16)
            nc.sync.dma_start(lhsB_sb[64:64 + K2, :], lhsB_d[:])

            div_b = dd_sb[:, 0:128].rearrange("p i -> p () i")
            t0 = 0
            for tpg in GROUP_PLAN:
                ramp = t0 < RAMP_TILES

                og = outp.tile([P, tpg, 2 * D], F32, tag="og")
                tg = angp.tile([P, tpg, 128], F32, tag="tg")

                # tg[p,t,i] = doy[p, t0+t] * div[i]
                doy_b = (
                    dd_sb[:, 128 + t0:128 + t0 + tpg]
                    .rearrange("p t -> p t ()")
                    .to_broadcast([P, tpg, 128])
                )
                nc.vector.tensor_tensor(
                    out=tg[:], in0=div_b.to_broadcast([P, tpg, 128]),
                    in1=doy_b, op=AOT.mult,
                )

                # range-reduce cols < R into [-pi, pi]:
                #   q = round(t/2pi);  t -= 2pi*q
                uc = angp.tile([P, tpg, R], F32, tag="uc")
                nc.vector.tensor_scalar(
                    out=uc[:], in0=tg[:, :, 0:R], scalar1=INV_2PI, scalar2=MAGIC,
                    op0=AOT.mult, op1=AOT.add,
                )
                nq = angp.tile([P, tpg, R], F32, tag="nq")
                nc.vector.tensor_scalar(
                    out=nq[:], in0=uc[:], scalar1=MAGIC, scalar2=-TWO_PI,
                    op0=AOT.subtract, op1=AOT.mult,
                )
                nc.vector.tensor_tensor(
                    out=tg[:, :, 0:R], in0=tg[:, :, 0:R], in1=nq[:], op=AOT.add
                )

                # PE half: Sin first (ramp-critical), then |y| for
                # cos(y) = sin(pi/2 - |y|) (Sin spline needs [-pi,pi])
                nc.scalar.activation(og[:, :, D::2], tg[:], ACT.Sin)
                ay = angp.tile([P, tpg, 128], F32, tag="ay")
                nc.scalar.activation(ay[:], tg[:], ACT.Abs)
                nc.scalar.activation(
                    og[:, :, D + 1::2], ay[:], ACT.Sin,
                    scale=-1.0, bias=halfpi[:],
                )
                if ramp:
                    # stream the PE half from the ACT sequencer right after
                    # the cos — no cross-engine wait on the issue path
                    nc.scalar.dma_start(
                        outv[:, t0:t0 + tpg, D:2 * D], og[:, :, D:2 * D]
                    )

                # obs half: tiles 0,1 are single matmuls (K=11); tiles 2+
                # go two-per-matmul with the block-diag stationary operand
                t = t0
                while t < t0 + tpg:
                    if t < 2:
                        ps = psump1.tile([P, D], F32, tag="ps1")
                        nc.tensor.matmul(
                            ps[:],
                            lhsA_sb[0:K, SINGLE_OFF + t * P:SINGLE_OFF + (t + 1) * P],
                            lhsA_sb[0:K, RHSA_OFF:RHSA_OFF + D],
                        )
                        nc.vector.tensor_copy(out=og[:, t - t0, 0:D], in_=ps[:])
                        t += 1
                        continue
                    pg = (t - 2) // 2
                    if pg < NPAIR_A:
                        lt = lhsA_sb[:, pg * P:(pg + 1) * P]
                        rhs = lhsA_sb[:, RHSA_OFF:]
                    else:
                        lt = lhsB_sb[64:64 + K2,
                                     (pg - NPAIR_A) * P:(pg - NPAIR_A + 1) * P]
                        rhs = lhsB_sb[64:64 + K2, RHSB_OFF:]
                    ps = psump.tile([P, 2 * D], F32, tag="ps")
                    nc.tensor.matmul(ps[:], lt, rhs)
                    src = ps[:].rearrange("p (t c) -> p t c", t=2)
                    dst = og[:, t - t0:t - t0 + 2, 0:D]
                    # fill phase: all copies on DVE so ACT stays on
                    # sin/abs/cos; steady: alternate DVE/ACT
                    if ramp or (pg % 2 == 1):
                        nc.vector.tensor_copy(out=dst, in_=src)
                    else:
                        nc.scalar.copy(dst, src)
                    t += 2

                if ramp:
                    nc.sync.dma_start(
                        outv[:, t0:t0 + tpg, 0:D], og[:, :, 0:D]
                    )
                else:
                    nc.sync.dma_start(outv[:, t0:t0 + tpg, :], og[:])
                t0 += tpg
    import os
    if not os.environ.get("KBUILD_DRY"):
        nc.compile()
    return nc


def _host_prep(input_sequence, doy_sequence, W, b):
    x = np.ascontiguousarray(np.asarray(input_sequence, dtype=np.float32))
    doy = np.asarray(doy_sequence)
    Wf = np.asarray(W, dtype=np.float32)
    bf = np.asarray(b, dtype=np.float32)

    # block-diagonal rhs [2K, 2D] in bf16
    rhs = np.zeros((K2, 2 * D), dtype=np.float32)
    rhs[:F, :D] = Wf.T
    rhs[F, :D] = bf
    rhs[K:K + F, D:] = Wf.T
    rhs[K + F, D:] = bf
    rhs = rhs.astype(ml_dtypes.bfloat16)

    div = np.exp(
        np.arange(0, D, 2, dtype=np.float32) * np.float32(-np.log(10000.0) / D)
    ).astype(np.float32)

    xs = x.reshape(N_CORES, TOK, F).astype(ml_dtypes.bfloat16)
    ds = doy.reshape(N_CORES, TOK).astype(np.float32)

    in_maps = []
    for c in range(N_CORES):
        xt = xs[c].reshape(G, P, F)          # [tile, p, f]
        # pairs over tiles 2..127, cols pair-major then lane
        pr = np.zeros((K2, NPAIR * P), dtype=ml_dtypes.bfloat16)
        pr[:F] = xt[2::2].transpose(2, 0, 1).reshape(F, NPAIR * P)
        pr[F] = 1.0
        pr[K:K + F] = xt[3::2].transpose(2, 0, 1).reshape(F, NPAIR * P)
        pr[K + F] = 1.0
        # single tiles 0,1 (K=11 rows)
        sg = np.zeros((K2, 2 * P), dtype=ml_dtypes.bfloat16)
        sg[:F] = xt[0:2].transpose(2, 0, 1).reshape(F, 2 * P)
        sg[F] = 1.0
        lhsA = np.concatenate([pr[:, :NPAIR_A * P], sg, rhs], axis=1)
        lhsB = np.concatenate([pr[:, NPAIR_A * P:], rhs], axis=1)
        dd = np.empty((P, 128 + G), dtype=np.float32)
        dd[:, 0:128] = div[None, :]
        dd[:, 128:] = ds[c].reshape(G, P).T
        in_maps.append({"lhsA": np.ascontiguousarray(lhsA),
                        "lhsB": np.ascontiguousarray(lhsB),
                        "dd": dd})
    return in_maps


def _get_nc():
    if "nc" not in _CACHE:
        _CACHE["nc"] = _build_nc()
    return _CACHE["nc"]


def kernel(input_sequence, doy_sequence, W, b, _trace=False, _trace_kwargs=None):
    from concourse.bass_utils import run_bass_kernel_spmd

    nc = _get_nc()
    in_maps = _host_prep(input_sequence, doy_sequence, W, b)
    kw = {}
    if _trace:
        kw.update(trace=True, **(_trace_kwargs or {}))
    res = run_bass_kernel_spmd(nc, in_maps, core_ids=list(range(N_CORES)), **kw)
    # out DRAM is partition-contiguous: [p, t*512:(t+1)*512] holds token
    # t*128+p of the core; unscramble on host
    out = np.empty((N_CORES, TOK, 2 * D), dtype=np.float32)
    for c in range(N_CORES):
        arr = res.results[c]["out"]
        out[c] = arr.reshape(P, G, 2 * D).transpose(1, 0, 2).reshape(TOK, 2 * D)
    out = out.reshape(B, S, 2 * D)
    if _trace:
        _CACHE["last_results"] = res
    return out


# revision 10
# speedup vs baseline: 1.2202x; 1.2202x over previous
"""BERT-embedding kernel for 8 Trainium2 NeuronCores (Bass/Tile).

out[b,s,:] = concat( input[b,s,:] @ W.T + b_vec,  PE[doy[b,s], :] )
with PE the standard sinusoidal table (d_model=256, max_len=366).

Strategy (data-parallel over batch, 8 cores), v2:
  - core c handles batches [c*128, (c+1)*128) = 16384 tokens.
  - The kernel is HBM-write-bound (~33.5 MB/core of output). Output DRAM
    layout is partition-contiguous ([128, G*512]; host unscrambles) so
    each group DMA moves 16 KB contiguous per partition -> large packets.
  - obs half: TensorE matmul in bf16 (rel err ~3.5e-3 << 2e-2 gate).
    Two token tiles per matmul via a block-diagonal stationary operand
    (K = 2*11 = 22, N = 512). lhs is split into two DRAM tensors loaded
    at partition offsets 0 and 64 so the input DMA spreads over both the
    even and odd SDMA ports instead of only the 6 even ones.
  - PE half: sin/cos(doy*div[i]) on ACT. Sin spline valid on [-pi,pi]:
    cols i < R are range-reduced with the f32 magic-number rounding trick
    on DVE. cos(y) = sin(pi/2 - |y|); |y| via DVE abs_max (keeps ACT free
    for Sin + table load in the ramp). The Sin ACT table is warmed at
    body start so the load overlaps the input DMAs.
  - angle inputs (div table + transposed doy) are packed into ONE DRAM
    tensor -> one DMA -> one completion semaphore on the ramp path.
  - ramp groups stream the PE half (issued from the ACT sequencer via
    scalar.dma_start) before matmul inputs land; obs half follows on SP.
"""
import numpy as np
import ml_dtypes

# ---------------- problem constants (hardcoded per contract) ----------------
B, S, F, D = 1024, 128, 10, 256
MAX_LEN = 366
N_CORES = 8
BPC = B // N_CORES          # batches per core
TOK = BPC * S               # tokens per core = 16384
P = 128                     # tokens per tile (SBUF partitions)
G = TOK // P                # 128 tiles per core
# group sizes (tiles per group): small leading groups so the first output
# DMAs fire early, then steady 8-tile groups
GROUP_PLAN = [2, 2, 4] + [8] * 15
assert sum(GROUP_PLAN) == G
RAMP_TILES = 8              # groups starting below this stream halves separately
K = F + 1                   # contraction dim incl. bias row
K2 = 2 * K                  # packed two-tile contraction dim
R = 68                      # columns needing range reduction (365*div[68] < pi)
NPAIR_A = 32                # pairs in lhsA (tiles 0..63); rest in lhsB @ part 64

PI = float(np.float32(np.pi))
HALF_PI = float(np.float32(np.pi / 2))
TWO_PI = float(np.float32(2 * np.pi))
INV_2PI = float(np.float32(1.0 / (2 * np.pi)))
MAGIC = 12582912.0          # 1.5 * 2**23: (x+MAGIC)-MAGIC == round-to-nearest(x)

_CACHE = {}


def _build_nc():
    import concourse.bacc as bacc
    import concourse.tile as tile
    import concourse.mybir as mybir

    F32 = mybir.dt.float32
    BF16 = mybir.dt.bfloat16
    AOT = mybir.AluOpType
    ACT = mybir.ActivationFunctionType

    nc = bacc.Bacc("TRN2", target_bir_lowering=False, debug=False,
                   num_devices=N_CORES)
    # lhsA: cols 0:NPAIR_A*P packed pairs (tiles 0..2*NPAIR_A-1), then 2D rhs
    # block-diag cols. lhsB: the remaining pairs + its own rhs copy (matmul
    # requires lhsT and rhs at the same base partition).
    NB = G // 2 - NPAIR_A
    lhsA_d = nc.dram_tensor("lhsA", [K2, NPAIR_A * P + 2 * D], BF16,
                            kind="ExternalInput")
    lhsB_d = nc.dram_tensor("lhsB", [K2, NB * P + 2 * D], BF16,
                            kind="ExternalInput")
    # dd: cols 0:128 = div row broadcast to all partitions, 128:128+G = doyT
    dd_d = nc.dram_tensor("dd", [P, 128 + G], F32, kind="ExternalInput")
    out_d = nc.dram_tensor("out", [P, G * 2 * D], F32, kind="ExternalOutput")

    outv = out_d[:].rearrange("p (t c) -> p t c", c=2 * D)

    with tile.TileContext(nc) as tc:
        with (
            tc.tile_pool(name="const", bufs=1) as cpool,
            tc.tile_pool(name="angp", bufs=3) as angp,
            tc.tile_pool(name="outp", bufs=5) as outp,
            tc.tile_pool(name="psum", bufs=6, space="PSUM") as psump,
        ):
            # one DMA, one completion sem for everything the PE half needs
            dd_sb = cpool.tile([P, 128 + G], F32)
            nc.sync.dma_start(dd_sb[:], dd_d[:])
            halfpi = cpool.tile([P, 1], F32)
            nc.vector.memset(halfpi[:], HALF_PI)
            # warm the Sin ACT table immediately so ACT_TABLE_LOAD overlaps
            # the input DMAs instead of gating the first PE-half group
            warm = cpool.tile([P, 1], F32)
            nc.scalar.activation(warm[:], halfpi[:], ACT.Sin)
            # matmul inputs: evens ports (partitions 0..21) and odd ports
            # (partitions 64..85)
            lhsA_sb = cpool.tile([K2, NPAIR_A * P + 2 * D], BF16)
            nc.sync.dma_start(lhsA_sb[:], lhsA_d[:])
            lhsB_sb = cpool.tile([64 + K2, NB * P + 2 * D], BF16)
            nc.sync.dma_start(lhsB_sb[64:64 + K2, :], lhsB_d[:])

            div_b = (
                dd_sb[:, 0:128].rearrange("p i -> p () i")
            )
            t0 = 0
            pair0 = 0
            for gi, tpg in enumerate(GROUP_PLAN):
                npair = tpg // 2
                ramp = t0 < RAMP_TILES

                og = outp.tile([P, tpg, 2 * D], F32, tag="og")
                tg = angp.tile([P, tpg, 128], F32, tag="tg")

                # tg[p,t,i] = doy[p, t0+t] * div[i]
                doy_b = (
                    dd_sb[:, 128 + t0:128 + t0 + tpg]
                    .rearrange("p t -> p t ()")
                    .to_broadcast([P, tpg, 128])
                )
                nc.vector.tensor_tensor(
                    out=tg[:], in0=div_b.to_broadcast([P, tpg, 128]),
                    in1=doy_b, op=AOT.mult,
                )

                # range-reduce cols < R into [-pi, pi]:
                #   q = round(t/2pi);  t -= 2pi*q
                uc = angp.tile([P, tpg, R], F32, tag="uc")
                nc.vector.tensor_scalar(
                    out=uc[:], in0=tg[:, :, 0:R], scalar1=INV_2PI, scalar2=MAGIC,
                    op0=AOT.mult, op1=AOT.add,
                )
                nq = angp.tile([P, tpg, R], F32, tag="nq")
                nc.vector.tensor_scalar(
                    out=nq[:], in0=uc[:], scalar1=MAGIC, scalar2=-TWO_PI,
                    op0=AOT.subtract, op1=AOT.mult,
                )
                nc.vector.tensor_tensor(
                    out=tg[:, :, 0:R], in0=tg[:, :, 0:R], in1=nq[:], op=AOT.add
                )
                # PE half first: interleaved sin/cos via ACT. Sin goes first
                # so the ramp-critical path isn't stuck behind Abs;
                # |y| for cos(y) = sin(pi/2 - |y|) (Sin spline needs [-pi,pi])
                nc.scalar.activation(og[:, :, D::2], tg[:], ACT.Sin)
                ay = angp.tile([P, tpg, 128], F32, tag="ay")
                nc.scalar.activation(ay[:], tg[:], ACT.Abs)
                nc.scalar.activation(
                    og[:, :, D + 1::2], ay[:], ACT.Sin,
                    scale=-1.0, bias=halfpi[:],
                )
                if ramp:
                    # stream the PE half from the ACT sequencer right after
                    # the cos — no cross-engine wait on the issue path
                    nc.scalar.dma_start(
                        outv[:, t0:t0 + tpg, D:2 * D], og[:, :, D:2 * D]
                    )

                # obs half: one matmul per token-tile pair (block-diag pack)
                for p2 in range(npair):
                    pg = pair0 + p2
                    if pg < NPAIR_A:
                        lt = lhsA_sb[:, pg * P:(pg + 1) * P]
                        rhs = lhsA_sb[:, NPAIR_A * P:]
                    else:
                        lt = lhsB_sb[64:64 + K2,
                                     (pg - NPAIR_A) * P:(pg - NPAIR_A + 1) * P]
                        rhs = lhsB_sb[64:64 + K2, NB * P:]
                    ps = psump.tile([P, 2 * D], F32, tag="ps")
                    nc.tensor.matmul(ps[:], lt, rhs)
                    src = ps[:].rearrange("p (t c) -> p t c", t=2)
                    dst = og[:, 2 * p2:2 * p2 + 2, 0:D]
                    # ramp: copies on ACT (after its sin/cos) keeps DVE free
                    # to race ahead on the next group's angles; steady: 50/50
                    if ramp or (pg % 2 == 0):
                        nc.scalar.copy(dst, src)
                    else:
                        nc.vector.tensor_copy(out=dst, in_=src)

                if ramp:
                    nc.sync.dma_start(
                        outv[:, t0:t0 + tpg, 0:D], og[:, :, 0:D]
                    )
                else:
                    nc.sync.dma_start(outv[:, t0:t0 + tpg, :], og[:])
                t0 += tpg
                pair0 += npair
    import os
    if not os.environ.get("KBUILD_DRY"):
        nc.compile()
    return nc


def _host_prep(input_sequence, doy_sequence, W, b):
    x = np.ascontiguousarray(np.asarray(input_sequence, dtype=np.float32))
    doy = np.asarray(doy_sequence)
    Wf = np.asarray(W, dtype=np.float32)
    bf = np.asarray(b, dtype=np.float32)

    # block-diagonal rhs [2K, 2D] in bf16
    rhs = np.zeros((K2, 2 * D), dtype=np.float32)
    rhs[:F, :D] = Wf.T
    rhs[F, :D] = bf
    rhs[K:K + F, D:] = Wf.T
    rhs[K + F, D:] = bf
    rhs = rhs.astype(ml_dtypes.bfloat16)

    div = np.exp(
        np.arange(0, D, 2, dtype=np.float32) * np.float32(-np.log(10000.0) / D)
    ).astype(np.float32)

    xs = x.reshape(N_CORES, TOK, F).astype(ml_dtypes.bfloat16)
    ds = doy.reshape(N_CORES, TOK).astype(np.float32)

    NB = G // 2 - NPAIR_A
    in_maps = []
    for c in range(N_CORES):
        # packed lhs: [2K, TOK/2]; tiles interleaved pairwise
        xt = xs[c].reshape(G, P, F)          # [tile, p, f]
        lhs = np.zeros((K2, TOK // 2), dtype=ml_dtypes.bfloat16)
        xt_even = xt[0::2]                   # [G/2, P, F]
        xt_odd = xt[1::2]
        # cols: pair-major then p
        lhs[:F] = xt_even.transpose(2, 0, 1).reshape(F, TOK // 2)
        lhs[F] = 1.0
        lhs[K:K + F] = xt_odd.transpose(2, 0, 1).reshape(F, TOK // 2)
        lhs[K + F] = 1.0
        lhsA = np.concatenate([lhs[:, :NPAIR_A * P], rhs], axis=1)
        lhsB = np.concatenate([lhs[:, NPAIR_A * P:], rhs], axis=1)
        dd = np.empty((P, 128 + G), dtype=np.float32)
        dd[:, 0:128] = div[None, :]
        dd[:, 128:] = ds[c].reshape(G, P).T
        in_maps.append({"lhsA": np.ascontiguousarray(lhsA),
                        "lhsB": np.ascontiguousarray(lhsB),
                        "dd": dd})
    return in_maps


def _get_nc():
    if "nc" not in _CACHE:
        _CACHE["nc"] = _build_nc()
    return _CACHE["nc"]


def kernel(input_sequence, doy_sequence, W, b, _trace=False, _trace_kwargs=None):
    from concourse.bass_utils import run_bass_kernel_spmd

    nc = _get_nc()
    in_maps = _host_prep(input_sequence, doy_sequence, W, b)
    kw = {}
    if _trace:
        kw.update(trace=True, **(_trace_kwargs or {}))
    res = run_bass_kernel_spmd(nc, in_maps, core_ids=list(range(N_CORES)), **kw)
    # out DRAM is partition-contiguous: [p, t*512:(t+1)*512] holds token
    # t*128+p of the core; unscramble on host
    out = np.empty((N_CORES, TOK, 2 * D), dtype=np.float32)
    for c in range(N_CORES):
        arr = res.results[c]["out"]
        out[c] = arr.reshape(P, G, 2 * D).transpose(1, 0, 2).reshape(TOK, 2 * D)
    out = out.reshape(B, S, 2 * D)
    if _trace:
        _CACHE["last_results"] = res
    return out
